# revision 4
# baseline (speedup 1.0000x reference)
"""Bayesian linear layer (Monte-Carlo reparameterized GEMM) on 8 Trainium2 cores.

y[s,b,o] = sum_i x[b,i] * (w_mu[o,i] + exp(w_lsigma[o,i]) * r1[s,o,i]) + b_mu[o]
           + exp(b_lsigma[o]) * r2[s,o]

Sharding: samples s split across the 8 cores (8 samples/core); x and the
(mu, lsigma) parameters replicated.

Per-core device kernel:
  - stream r1[s] tiles, PE-transpose them, fuse  w_sT = E^T * r1^T + w_mu^T
    on DVE (constants resident in SBUF in [i,o] layout)
  - GEMM y[s] = x @ w_s^T as float32r (FP22) matmuls: lhsT = x^T tiles
    (streamed, shared across a sample pair), rhs = w_sT, accumulate over
    k in PSUM
  - evict PSUM via ACT copy + DVE bias add, DMA to DRAM
"""

import sys

if "/opt/trn_rl_repo" not in sys.path:
    sys.path.insert(0, "/opt/trn_rl_repo")

from contextlib import ExitStack

import numpy as np

import concourse.bass as bass  # noqa: F401
import concourse.tile as tile
from concourse import bacc, mybir
from concourse.bass_utils import run_bass_kernel_spmd
from concourse.masks import make_identity

P = 128
N_IN = 1024
N_OUT = 1024
BATCH = 4096
S = 64
NCORES = 8
SC = S // NCORES  # samples per core
KT = N_IN // P  # 8 k-tiles
BT = BATCH // P  # 32 b-tiles
OW = 512  # o chunk (one PSUM bank of fp32)
OH = N_OUT // OW  # 2 o-halves

F32 = mybir.dt.float32
F32R = mybir.dt.float32r

_CACHE = {}


def build_bass():
    nc = bacc.Bacc("TRN2", target_bir_lowering=False, debug=False)

    xT = nc.dram_tensor("xT", [N_IN, BATCH], F32, kind="ExternalInput").ap()
    ET = nc.dram_tensor("ET", [N_IN, N_OUT], F32, kind="ExternalInput").ap()
    wmuT = nc.dram_tensor("wmuT", [N_IN, N_OUT], F32, kind="ExternalInput").ap()
    r1s = nc.dram_tensor("r1s", [SC, N_OUT, N_IN], F32, kind="ExternalInput").ap()
    biass = nc.dram_tensor("biass", [SC, N_OUT], F32, kind="ExternalInput").ap()
    y = nc.dram_tensor("y", [SC, BATCH, N_OUT], F32, kind="ExternalOutput").ap()

    with tile.TileContext(nc) as tc, ExitStack() as ctx:
        const = ctx.enter_context(tc.tile_pool(name="const", bufs=1))
        xt_pool = ctx.enter_context(tc.tile_pool(name="xt", bufs=3))
        wst_pool = ctx.enter_context(tc.tile_pool(name="wst", bufs=2))
        r1_pool = ctx.enter_context(tc.tile_pool(name="r1", bufs=2))
        y_pool = ctx.enter_context(tc.tile_pool(name="yp", bufs=4))
        bias_pool = ctx.enter_context(tc.tile_pool(name="bias", bufs=2))
        bias1_pool = ctx.enter_context(tc.tile_pool(name="bias1", bufs=2))
        pt_pool = ctx.enter_context(tc.tile_pool(name="pt", bufs=2, space="PSUM"))
        pm_pool = ctx.enter_context(tc.tile_pool(name="pm", bufs=6, space="PSUM"))

        ident = const.tile([P, P], F32)
        make_identity(nc, ident[:])

        # constants resident in [i, o] layout: [p, k, o] with i = k*P + p
        ET_sb = const.tile([P, KT, N_OUT], F32)
        wmuT_sb = const.tile([P, KT, N_OUT], F32)
        for k in range(KT):
            nc.sync.dma_start(ET_sb[:, k, :], ET[k * P : (k + 1) * P, :])
            nc.sync.dma_start(wmuT_sb[:, k, :], wmuT[k * P : (k + 1) * P, :])

        def transform(s):
            """Build w_sT for sample s: [p, k, o] with w_sT[p,k,o] = w_s[o, k*P+p]."""
            wst = wst_pool.tile([P, KT, N_OUT], F32R, tag="wst")
            for oh in range(OH):
                osl = slice(oh * OW, (oh + 1) * OW)
                slab = r1_pool.tile([P, 4, N_IN], F32, tag="r1")
                for ot in range(4):
                    nc.sync.dma_start(
                        slab[:, ot, :],
                        r1s[s, oh * OW + ot * P : oh * OW + (ot + 1) * P, :],
                    )
                for it in range(KT):
                    ps = pt_pool.tile([P, OW], F32, tag="pt")
                    for ot in range(4):
                        nc.tensor.transpose(
                            ps[:, ot * P : (ot + 1) * P],
                            slab[:, ot, it * P : (it + 1) * P],
                            ident[:],
                        )
                    nc.vector.tensor_mul(wst[:, it, osl], ps[:], ET_sb[:, it, osl])
                    nc.vector.tensor_add(
                        wst[:, it, osl], wst[:, it, osl], wmuT_sb[:, it, osl]
                    )
            return wst

        def load_bias(s):
            b1 = bias1_pool.tile([1, N_OUT], F32, tag="b1")
            nc.sync.dma_start(b1[:], biass[s][None, :])
            bm = bias_pool.tile([P, N_OUT], F32, tag="bias")
            nc.gpsimd.partition_broadcast(bm[:], b1[:])
            return bm

        for pr in range(SC // 2):
            pair = (2 * pr, 2 * pr + 1)
            ws = {s: transform(s) for s in pair}
            bm = {s: load_bias(s) for s in pair}
            for bt in range(BT):
                xt = xt_pool.tile([P, KT, P], F32R, tag="xt")
                xslab = xT[:, bt * P : (bt + 1) * P].rearrange("(k p) b -> p k b", p=P)
                for kh in range(4):  # split DMA across queues
                    nc.sync.dma_start(xt[:, 2 * kh : 2 * kh + 2, :], xslab[:, 2 * kh : 2 * kh + 2, :].bitcast(F32R))
                # 4 accumulation groups (2 samples x 2 o-halves), k-major so the
                # stationary x tile is reused between consecutive matmuls
                pms = {}
                for s in pair:
                    for oh in range(OH):
                        pms[(s, oh)] = pm_pool.tile([P, OW], F32, tag="pm", name=f"pm_{s}_{oh}")
                for k in range(KT):
                    lhsT = xt[:, k, :]
                    for s in pair:
                        for oh in range(OH):
                            nc.tensor.matmul(
                                pms[(s, oh)][:],
                                lhsT,
                                ws[s][:, k, oh * OW : (oh + 1) * OW],
                                start=(k == 0),
                                stop=(k == KT - 1),
                            )
                for s in pair:
                    for oh in range(OH):
                        osl = slice(oh * OW, (oh + 1) * OW)
                        yt = y_pool.tile([P, OW], F32, tag="y")
                        nc.scalar.copy(yt[:], pms[(s, oh)][:])
                        nc.vector.tensor_add(yt[:], yt[:], bm[s][:, osl])
                        nc.sync.dma_start(y[s, bt * P : (bt + 1) * P, osl], yt[:])

    nc.compile()
    return nc


def _prep(x, w_mu, w_lsigma, b_mu, b_lsigma, r1, r2):
    xT = np.ascontiguousarray(x.T)
    ET = np.ascontiguousarray(np.exp(w_lsigma).T.astype(np.float32))
    wmuT = np.ascontiguousarray(w_mu.T)
    bias = (b_mu[None, :] + np.exp(b_lsigma)[None, :] * r2).astype(np.float32)
    return xT, ET, wmuT, bias


def kernel(x, w_mu, w_lsigma, b_mu, b_lsigma, r1, r2, N_samples):
    x = np.asarray(x, dtype=np.float32)
    w_mu = np.asarray(w_mu, dtype=np.float32)
    w_lsigma = np.asarray(w_lsigma, dtype=np.float32)
    b_mu = np.asarray(b_mu, dtype=np.float32)
    b_lsigma = np.asarray(b_lsigma, dtype=np.float32)
    r1 = np.asarray(r1, dtype=np.float32)
    r2 = np.asarray(r2, dtype=np.float32)
    assert x.shape == (BATCH, N_IN) and r1.shape == (S, N_OUT, N_IN)

    if "nc" not in _CACHE:
        _CACHE["nc"] = build_bass()
    nc = _CACHE["nc"]

    xT, ET, wmuT, bias = _prep(x, w_mu, w_lsigma, b_mu, b_lsigma, r1, r2)

    in_maps = []
    for c in range(NCORES):
        sl = slice(c * SC, (c + 1) * SC)
        in_maps.append(
            {
                "xT": xT,
                "ET": ET,
                "wmuT": wmuT,
                "r1s": np.ascontiguousarray(r1[sl]),
                "biass": np.ascontiguousarray(bias[sl]),
            }
        )

    res = run_bass_kernel_spmd(nc, in_maps, core_ids=list(range(NCORES)))
    out = np.concatenate([res.results[c]["y"] for c in range(NCORES)], axis=0)
    return out
